# revision 12
# baseline (speedup 1.0000x reference)
"""Trainium2 Bass kernel for nn_Network_51445118271910 (moe_routing).

Math (identical to the reference, dead-code-eliminated):
  - The reference returns (probs[0], 0.0); every op in the network is
    batch-independent, so only batch row 0 of x matters.
  - o = emb[x0] + pos_encoding            (256, 512)  on-device gather
  - 2x MHSA, 8 heads, no residual         (256, 512)
  - h = relu(o.flat @ w1 + b1)            (512,)
  - z = h @ w2 + b2                       (32000,)
  - out = softmax(softmax(z))             (host epilogue, 32000 floats)

Sharding over 8 cores (single SPMD launch, no device collectives):
  - attention is replicated (tiny: one sequence),
  - w1 (131072x512) column-sharded: core c computes h[64c:64c+64],
  - w2 (512x32000) row-sharded: core c computes the partial logits
    h[64c:64c+64] @ w2[64c:64c+64, :] -> (32000,),
  - host sums the 8 partial logit vectors, adds b2, double-softmax.
"""

import numpy as np

import concourse.bass as bass
import concourse.mybir as mybir
import concourse.tile as tile
from concourse import bacc
from concourse.bass_utils import run_bass_kernel_spmd
from concourse.masks import make_identity

P = 128
SEQ = 256
HID = 512
NH = 8
DK = 64
NL = 2
VOCAB = 16384
OUT = 32000
KT = HID // P          # 4 k-tiles of the hidden dim
ST = SEQ // P          # 2 tiles of the sequence dim
NCORES = 8
WCOL = 64              # w1 columns / w2 rows per core
HALF_OUT = OUT // 2    # 16000
ZT = HALF_OUT // P     # 125 z column-blocks per half
W2_CHUNK_T = 25        # z column-blocks per streamed w2 chunk
W2_CHUNKS = ZT // W2_CHUNK_T
W1_G = 8               # tokens per streamed w1 tile
F32 = mybir.dt.float32
AF = mybir.ActivationFunctionType


def _build_nc():
    nc = bacc.Bacc("TRN2", target_bir_lowering=False, debug=False,
                   num_devices=NCORES)

    xidx = nc.dram_tensor("xidx", [P, ST], mybir.dt.int32, kind="ExternalInput").ap()
    emb = nc.dram_tensor("emb", [VOCAB, HID], F32, kind="ExternalInput").ap()
    pos = nc.dram_tensor("pos", [P, ST, HID], F32, kind="ExternalInput").ap()
    wqkv = nc.dram_tensor("wqkv", [P, NL, 3, KT, HID], F32, kind="ExternalInput").ap()
    w1c = nc.dram_tensor("w1c", [P, SEQ, KT * WCOL], F32, kind="ExternalInput").ap()
    b1c = nc.dram_tensor("b1c", [P, 1], F32, kind="ExternalInput").ap()
    w2c = nc.dram_tensor("w2c", [P, HALF_OUT], F32, kind="ExternalInput").ap()
    zout = nc.dram_tensor("zout", [P, 2 * ZT], F32, kind="ExternalOutput").ap()

    with tile.TileContext(nc) as tc:
        with (
            tc.tile_pool(name="cp", bufs=1) as cp,          # constants / persistent
            tc.tile_pool(name="op", bufs=2) as op,          # o / oT activations
            tc.tile_pool(name="qk", bufs=2) as qk,          # qT / kT / v
            tc.tile_pool(name="ep", bufs=3) as ep,          # exp(scores^T) per head
            tc.tile_pool(name="rp", bufs=4) as rp,          # tiny per-head scalars
            tc.tile_pool(name="w1p", bufs=4) as w1p,        # streamed w1 tiles
            tc.tile_pool(name="w2p", bufs=2) as w2p,        # streamed w2 chunks
            tc.tile_pool(name="psA", bufs=3, space="PSUM") as psA,   # matmul banks
            tc.tile_pool(name="psB", bufs=2, space="PSUM") as psB,   # attention out
            tc.tile_pool(name="psC", bufs=2, space="PSUM") as psC,   # transposes / z
        ):
            # ---- constants & inputs resident in SBUF ----
            ident = cp.tile([P, P], F32, tag="ident")
            make_identity(nc, ident)
            ones = cp.tile([P, 1], F32, tag="ones")
            nc.gpsimd.memset(ones, 1.0)
            x_sb = cp.tile([P, ST], mybir.dt.int32, tag="x")
            nc.sync.dma_start(x_sb, xidx)
            b1_sb = cp.tile([P, 1], F32, tag="b1")
            nc.sync.dma_start(b1_sb, b1c)


            w_sb = {}
            for l in range(NL):
                for m in range(3):
                    t = cp.tile([P, KT, HID], F32, tag=f"w{l}{m}")
                    nc.sync.dma_start(t, wqkv[:, l, m])
                    w_sb[(l, m)] = t

            # ---- embedding gather + positional encoding ----
            # pos is DMAed into o0 first, then the gather accumulates emb[x0]
            # on top of it (CCE add in the DMA engine).
            o0 = op.tile([P, ST, HID], F32, tag="o")
            nc.gpsimd.dma_start(o0, pos)
            for i in range(ST):
                nc.gpsimd.indirect_dma_start(
                    out=o0[:, i, :], out_offset=None, in_=emb,
                    in_offset=bass.IndirectOffsetOnAxis(ap=x_sb[:, i:i + 1], axis=0),
                    compute_op=mybir.AluOpType.add,
                )

            def transpose_sd_to_ds(o_sd, oT_ds):
                # [s-part, st, d] -> [d-part, dt, s] via 8 PE transposes
                for dt in range(KT):
                    for st in range(ST):
                        pt = psC.tile([P, P], F32, tag="tr")
                        nc.tensor.transpose(pt, o_sd[:, st, dt * P:(dt + 1) * P], ident)
                        nc.any.tensor_copy(oT_ds[:, dt, st * P:(st + 1) * P], pt)

            oT = op.tile([P, KT, SEQ], F32, tag="oT")
            transpose_sd_to_ds(o0, oT)

            # ---- 2 MHSA layers ----
            for l in range(NL):
                qT = qk.tile([P, KT, SEQ], F32, tag="qT")
                kT = qk.tile([P, KT, SEQ], F32, tag="kT")
                # v with a ones-column per head (for the softmax denominator)
                v_sb = qk.tile([P, ST, NH, DK + 1], F32, tag="v")
                nc.gpsimd.memset(v_sb[:, :, :, DK:DK + 1], 1.0)

                for m, dst in ((0, qT), (1, kT)):
                    for jt in range(KT):
                        pq = psA.tile([P, 512], F32, tag="mm")
                        for kt in range(KT):
                            nc.tensor.matmul(
                                pq[:, :SEQ],
                                lhsT=w_sb[(l, m)][:, kt, jt * P:(jt + 1) * P],
                                rhs=oT[:, kt, :],
                                start=(kt == 0), stop=(kt == KT - 1),
                            )
                        nc.any.tensor_copy(dst[:, jt, :], pq[:, :SEQ])

                for tt in range(ST):
                    pv = psA.tile([P, 512], F32, tag="mm")
                    for kt in range(KT):
                        nc.tensor.matmul(
                            pv,
                            lhsT=oT[:, kt, tt * P:(tt + 1) * P],
                            rhs=w_sb[(l, 2)][:, kt, :],
                            start=(kt == 0), stop=(kt == KT - 1),
                        )
                    for h in range(NH):
                        nc.any.tensor_copy(v_sb[:, tt, h, 0:DK],
                                           pv[:, h * DK:(h + 1) * DK])

                o_out = op.tile([P, ST, HID], F32, tag="o")
                for h in range(NH):
                    jt, lo = h // 2, (h % 2) * DK
                    qTh = qT[lo:lo + DK, jt, :]
                    kTh = kT[lo:lo + DK, jt, :]
                    eT = ep.tile([P, ST, SEQ], F32, tag="expT")
                    for tt in range(ST):
                        ps = psA.tile([P, 512], F32, tag="mm")
                        nc.tensor.matmul(ps[:, :SEQ],
                                         lhsT=kTh[:, tt * P:(tt + 1) * P],
                                         rhs=qTh, start=True, stop=True)
                        # softmax without max-subtraction: |scores|/8 <= ~2.2
                        nc.scalar.activation(eT[:, tt, :], ps[:, :SEQ],
                                             AF.Exp, scale=0.125)
                    for st in range(ST):
                        pa = psB.tile([P, DK + 1], F32, tag="att")
                        for tt in range(ST):
                            nc.tensor.matmul(pa,
                                             lhsT=eT[:, tt, st * P:(st + 1) * P],
                                             rhs=v_sb[:, tt, h, :],
                                             start=(tt == 0), stop=(tt == ST - 1))
                        rec = rp.tile([P, 1], F32, tag="rec")
                        nc.vector.reciprocal(rec, pa[:, DK:DK + 1])
                        nc.vector.tensor_scalar_mul(
                            o_out[:, st, h * DK:(h + 1) * DK], pa[:, 0:DK], rec)

                oT = op.tile([P, KT, SEQ], F32, tag="oT")
                transpose_sd_to_ds(o_out, oT)

            # ---- h = relu(flat @ w1 + b1), this core's 64 columns ----
            # per token s: psum[a, 64a+n] += sum_p oT[p, a, s] * w1[512s+128a+p, n]
            pw = psA.tile([4, KT * WCOL], F32, tag="mm")
            for g in range(SEQ // W1_G):
                wt = w1p.tile([P, W1_G, KT * WCOL], F32, tag="w1t")
                nc.sync.dma_start(wt, w1c[:, g * W1_G:(g + 1) * W1_G, :])
                for t in range(W1_G):
                    s = g * W1_G + t
                    nc.tensor.matmul(pw, lhsT=oT[:, :, s], rhs=wt[:, t, :],
                                     start=(s == 0), stop=(s == SEQ - 1))

            # diagonal extraction: h[n] = sum_a pw[a, 64a+n].  Four accumulating
            # K=4 matmuls, each selecting row a via an identity column; the
            # broadcast free dim duplicates h to both partition halves for the
            # two w2 K-groups.
            hsum = cp.tile([4, KT, P], F32, tag="hsum")
            for a in range(4):
                nc.any.tensor_copy(hsum[:, a, 0:DK], pw[:, a * DK:(a + 1) * DK])
                nc.any.tensor_copy(hsum[:, a, DK:P], pw[:, a * DK:(a + 1) * DK])
            ph = psC.tile([P, 1], F32, tag="tr")
            for a in range(4):
                nc.tensor.matmul(ph, lhsT=hsum[0:4, a, :], rhs=ident[0:4, a:a + 1],
                                 start=(a == 0), stop=(a == 3))
            h2 = cp.tile([P, 1], F32, tag="h2")
            nc.scalar.activation(h2, ph, AF.Relu, bias=b1_sb)

            # ---- partial z = h_c @ w2_c (this core's 64 rows of w2) ----
            # w2c is host-packed [128, 16000]: partitions 0:64 cover z columns
            # [0,16000), partitions 64:128 cover [16000,32000).
            pz = [psC.tile([P, ZT], F32, tag="tr", name=f"pz{half}")
                  for half in range(2)]
            for i in range(W2_CHUNKS):
                w2t = w2p.tile([P, W2_CHUNK_T * P], F32, tag="w2t")
                nc.sync.dma_start(w2t, w2c[:, i * W2_CHUNK_T * P:(i + 1) * W2_CHUNK_T * P])
                for half in range(2):
                    lo = half * DK
                    for lt in range(W2_CHUNK_T):
                        t = i * W2_CHUNK_T + lt
                        nc.tensor.matmul(pz[half][:, t:t + 1],
                                         lhsT=w2t[lo:lo + DK, lt * P:(lt + 1) * P],
                                         rhs=h2[lo:lo + DK, :],
                                         start=True, stop=True)
            z_sb = cp.tile([P, 2 * ZT], F32, tag="z")
            for half in range(2):
                nc.any.tensor_copy(z_sb[:, half * ZT:(half + 1) * ZT], pz[half])
            nc.sync.dma_start(zout, z_sb)

    nc.compile()
    return nc


_NC_CACHE = {}


def _get_nc():
    if "nc" not in _NC_CACHE:
        _NC_CACHE["nc"] = _build_nc()
    return _NC_CACHE["nc"]


def _pos_encoding_np():
    pos = np.arange(SEQ, dtype=np.float32)[:, None]
    div = np.power(np.float32(10000.0),
                   np.arange(0, HID, 2, dtype=np.float32) / np.float32(HID))
    ang = (pos / div).astype(np.float32)
    pe = np.zeros((SEQ, HID), np.float32)
    pe[:, 0::2] = np.sin(ang)
    pe[:, 1::2] = np.cos(ang)
    return pe


def _pack_inputs(x, emb, wq, wk, wv, w1, b1, w2):
    x0 = np.asarray(x)[0].astype(np.int32)
    xidx = np.ascontiguousarray(x0.reshape(ST, P).T)                     # [128, 2]
    posP = np.ascontiguousarray(
        _pos_encoding_np().reshape(ST, P, HID).transpose(1, 0, 2))       # [128, 2, 512]
    wqkv = np.stack([np.asarray(wq), np.asarray(wk), np.asarray(wv)], axis=1)
    wqkvP = np.ascontiguousarray(
        wqkv.reshape(NL, 3, KT, P, HID).transpose(3, 0, 1, 2, 4)
    ).astype(np.float32)                                                 # [128,2,3,4,512]
    embF = np.ascontiguousarray(np.asarray(emb), dtype=np.float32)
    w1F = np.asarray(w1, dtype=np.float32)
    w2F = np.asarray(w2, dtype=np.float32)
    b1F = np.asarray(b1, dtype=np.float32)

    in_maps = []
    for c in range(NCORES):
        w1c = w1F[:, c * WCOL:(c + 1) * WCOL]                            # (131072, 64)
        w1P = np.ascontiguousarray(
            w1c.reshape(SEQ, KT, P, WCOL).transpose(2, 0, 1, 3)
            .reshape(P, SEQ, KT * WCOL))                                 # [128, 256, 256]
        w2r = w2F[c * WCOL:(c + 1) * WCOL]                               # (64, 32000)
        w2P = np.ascontiguousarray(
            np.concatenate([w2r[:, :HALF_OUT], w2r[:, HALF_OUT:]], axis=0))  # [128,16000]
        b1P = np.ascontiguousarray(
            np.tile(b1F[c * WCOL:(c + 1) * WCOL], 2)[:, None])           # [128, 1]
        in_maps.append({
            "xidx": xidx, "emb": embF, "pos": posP, "wqkv": wqkvP,
            "w1c": w1P, "b1c": b1P, "w2c": w2P,
        })
    return in_maps


def run_on_device(in_maps, **kwargs):
    nc = _get_nc()
    return run_bass_kernel_spmd(nc, in_maps, list(range(NCORES)), **kwargs)


def _epilogue(results, b2):
    z = np.zeros(OUT, np.float64)
    for c in range(NCORES):
        zc = np.asarray(results[c]["zout"], dtype=np.float64)            # [128, 250]
        z[:HALF_OUT] += zc[:, :ZT].T.ravel()
        z[HALF_OUT:] += zc[:, ZT:].T.ravel()
    z += np.asarray(b2, dtype=np.float64)
    e = np.exp(z - z.max())
    g = e / e.sum()
    e2 = np.exp(g - g.max())
    probs = e2 / e2.sum()
    return probs.astype(np.float32)


def kernel(x, emb, wq, wk, wv, w1, b1, w2, b2):
    in_maps = _pack_inputs(x, emb, wq, wk, wv, w1, b1, w2)
    res = run_on_device(in_maps)
    probs = _epilogue(res.results, b2)
    return probs, np.float32(0.0)


# revision 17
# speedup vs baseline: 2.9957x; 2.9957x over previous
"""Trainium2 Bass kernel for nn_Network_51445118271910 (moe_routing).

Math (identical to the reference, dead-code-eliminated):
  - The reference returns (probs[0], 0.0); every op in the network is
    batch-independent, so only batch row 0 of x matters.
  - o = emb[x0] + pos_encoding            (256, 512)  on-device gather
  - 2x MHSA, 8 heads, no residual         (256, 512)
  - h = relu(o.flat @ w1 + b1)            (512,)
  - z = h @ w2 + b2                       (32000,)
  - out = softmax(softmax(z))             (host epilogue, 32000 floats)

Sharding over 8 cores (single SPMD launch, no device collectives):
  - attention is replicated (tiny: one sequence),
  - w1 (131072x512) column-sharded: core c computes h[64c:64c+64],
  - w2 (512x32000) row-sharded: core c computes the partial logits
    h[64c:64c+64] @ w2[64c:64c+64, :] -> (32000,),
  - host sums the 8 partial logit vectors, adds b2, double-softmax.

Weights stream in bf16 (ample precision: the double softmax at the end
makes the output insensitive at the 1e-4 level); PSUM accumulation is
fp32 throughout; partial logits return in fp32.
"""

import ml_dtypes
import numpy as np

import concourse.bass as bass
import concourse.mybir as mybir
import concourse.tile as tile
from concourse import bacc
from concourse.bass_utils import run_bass_kernel_spmd
from concourse.masks import make_identity

P = 128
SEQ = 256
HID = 512
NH = 8
DK = 64
NL = 2
VOCAB = 16384
OUT = 32000
KT = HID // P          # 4 k-tiles of the hidden dim
ST = SEQ // P          # 2 tiles of the sequence dim
NCORES = 8
WCOL = 64              # w1 columns / w2 rows per core
HALF_OUT = OUT // 2    # 16000
ZT = HALF_OUT // P     # 125 z column-blocks per half
NPAIR = SEQ // 2       # 128 token pairs for the w1 contraction
W1_G = 8               # token-pairs per streamed w1 tile (8*512*2B*128 = 1MB)
W1_BUFS = 14
F32 = mybir.dt.float32
BF16 = mybir.dt.bfloat16
AF = mybir.ActivationFunctionType
NP_BF16 = ml_dtypes.bfloat16


def _build_nc():
    nc = bacc.Bacc("TRN2", target_bir_lowering=False, debug=False,
                   num_devices=NCORES)

    xidx = nc.dram_tensor("xidx", [P, ST], mybir.dt.int32, kind="ExternalInput").ap()
    emb = nc.dram_tensor("emb", [VOCAB, HID], F32, kind="ExternalInput").ap()
    pos = nc.dram_tensor("pos", [P, ST, HID], F32, kind="ExternalInput").ap()
    wqkv = nc.dram_tensor("wqkv", [P, NL, 3, KT, HID], BF16, kind="ExternalInput").ap()
    w1c = nc.dram_tensor("w1c", [P, SEQ, KT * WCOL], BF16,
                         kind="ExternalInput").ap()
    b1c = nc.dram_tensor("b1c", [P, 1], F32, kind="ExternalInput").ap()
    w2c = nc.dram_tensor("w2c", [P, HALF_OUT], BF16, kind="ExternalInput").ap()
    zout = nc.dram_tensor("zout", [P, 2 * ZT], F32, kind="ExternalOutput").ap()

    with tile.TileContext(nc) as tc:
        with (
            tc.tile_pool(name="cp", bufs=1) as cp,          # constants / persistent
            tc.tile_pool(name="op", bufs=2) as op,          # o / oT activations
            tc.tile_pool(name="qk", bufs=1) as qk,          # qT / kT / v
            tc.tile_pool(name="ep", bufs=2) as ep,          # exp(scores^T) per head
            tc.tile_pool(name="rp", bufs=4) as rp,          # tiny per-head scalars
            tc.tile_pool(name="w1p", bufs=W1_BUFS) as w1p,  # streamed w1 tiles
            tc.tile_pool(name="psA", bufs=4, space="PSUM") as psA,   # matmul banks
            tc.tile_pool(name="psB", bufs=2, space="PSUM") as psB,   # attention out
            tc.tile_pool(name="psC", bufs=2, space="PSUM") as psC,   # transposes / z
        ):
            # ---- constants & small inputs ----
            ident = cp.tile([P, P], F32, tag="ident")
            make_identity(nc, ident)
            identb = cp.tile([P, P], BF16, tag="identb")
            make_identity(nc, identb)
            x_sb = cp.tile([P, ST], mybir.dt.int32, tag="x")
            nc.sync.dma_start(x_sb, xidx)
            b1_sb = cp.tile([P, 1], F32, tag="b1")
            nc.sync.dma_start(b1_sb, b1c)

            w_sb = {}
            for l in range(NL):
                for m in range(3):
                    t = cp.tile([P, KT, HID], BF16, tag=f"w{l}{m}")
                    nc.sync.dma_start(t, wqkv[:, l, m])
                    w_sb[(l, m)] = t

            # w2 resident (bf16, 4MB); streamed on the ACT HW-DGE ring so it
            # overlaps the w1 stream on the sync ring.
            w2_sb = cp.tile([P, HALF_OUT], BF16, tag="w2")
            for i in range(4):
                sl = slice(i * HALF_OUT // 4, (i + 1) * HALF_OUT // 4)
                nc.scalar.dma_start(w2_sb[:, sl], w2c[:, sl])

            # ---- embedding gather + positional encoding ----
            # pos is DMAed into o0 first, then the gather accumulates emb[x0]
            # on top of it (CCE add in the DMA engine).
            o0 = op.tile([P, ST, HID], F32, tag="o")
            nc.gpsimd.dma_start(o0, pos)
            for i in range(ST):
                nc.gpsimd.indirect_dma_start(
                    out=o0[:, i, :], out_offset=None, in_=emb,
                    in_offset=bass.IndirectOffsetOnAxis(ap=x_sb[:, i:i + 1], axis=0),
                    compute_op=mybir.AluOpType.add,
                )

            def transpose_sd_to_ds(o_sd, oT_ds, idt):
                # [s-part, st, d] -> [d-part, dt, s] via 8 PE transposes
                for dt in range(KT):
                    for st in range(ST):
                        pt = psC.tile([P, P], o_sd.dtype, tag="tr")
                        nc.tensor.transpose(pt, o_sd[:, st, dt * P:(dt + 1) * P], idt)
                        nc.any.tensor_copy(oT_ds[:, dt, st * P:(st + 1) * P], pt)

            oT = op.tile([P, KT, SEQ], BF16, tag="oT")
            transpose_sd_to_ds(o0, oT, ident)

            # ---- 2 MHSA layers (bf16 operands, fp32 PSUM) ----
            for l in range(NL):
                qT = qk.tile([P, KT, SEQ], BF16, tag="qT")
                kT = qk.tile([P, KT, SEQ], BF16, tag="kT")
                # v with a ones-column per head (for the softmax denominator)
                v_sb = qk.tile([P, ST, NH, DK + 1], BF16, tag="v")
                nc.gpsimd.memset(v_sb[:, :, :, DK:DK + 1], 1.0)

                for m, dst in ((0, qT), (1, kT)):
                    for jt in range(KT):
                        pq = psA.tile([P, 512], F32, tag="mm")
                        for kt in range(KT):
                            nc.tensor.matmul(
                                pq[:, :SEQ],
                                lhsT=w_sb[(l, m)][:, kt, jt * P:(jt + 1) * P],
                                rhs=oT[:, kt, :],
                                start=(kt == 0), stop=(kt == KT - 1),
                            )
                        nc.any.tensor_copy(dst[:, jt, :], pq[:, :SEQ])

                for tt in range(ST):
                    pv = psA.tile([P, 512], F32, tag="mm")
                    for kt in range(KT):
                        nc.tensor.matmul(
                            pv,
                            lhsT=oT[:, kt, tt * P:(tt + 1) * P],
                            rhs=w_sb[(l, 2)][:, kt, :],
                            start=(kt == 0), stop=(kt == KT - 1),
                        )
                    for h in range(NH):
                        nc.any.tensor_copy(v_sb[:, tt, h, 0:DK],
                                           pv[:, h * DK:(h + 1) * DK])

                o_out = op.tile([P, ST, HID], BF16, tag="ob")
                for h in range(NH):
                    jt, lo = h // 2, (h % 2) * DK
                    qTh = qT[lo:lo + DK, jt, :]
                    kTh = kT[lo:lo + DK, jt, :]
                    eT = ep.tile([P, ST, SEQ], BF16, tag="expT")
                    for tt in range(ST):
                        ps = psA.tile([P, 512], F32, tag="mm")
                        nc.tensor.matmul(ps[:, :SEQ],
                                         lhsT=kTh[:, tt * P:(tt + 1) * P],
                                         rhs=qTh, start=True, stop=True)
                        # softmax without max-subtraction: |scores|/8 <= ~2.2
                        nc.scalar.activation(eT[:, tt, :], ps[:, :SEQ],
                                             AF.Exp, scale=0.125)
                    for st in range(ST):
                        pa = psB.tile([P, DK + 1], F32, tag="att")
                        for tt in range(ST):
                            nc.tensor.matmul(pa,
                                             lhsT=eT[:, tt, st * P:(st + 1) * P],
                                             rhs=v_sb[:, tt, h, :],
                                             start=(tt == 0), stop=(tt == ST - 1))
                        rec = rp.tile([P, 1], F32, tag="rec")
                        nc.vector.reciprocal(rec, pa[:, DK:DK + 1])
                        nc.vector.tensor_scalar_mul(
                            o_out[:, st, h * DK:(h + 1) * DK], pa[:, 0:DK], rec)

                oT = op.tile([P, KT, SEQ], BF16, tag="oT")
                transpose_sd_to_ds(o_out, oT, identb)

            # ---- h = relu(flat @ w1 + b1), this core's 64 columns ----
            # per token s: psum[a, 64a+n] += sum_p oT[p, a, s] * w1[512s+128a+p, n]
            pw = psA.tile([4, KT * WCOL], F32, tag="mm")
            for g in range(SEQ // W1_G):
                wt = w1p.tile([P, W1_G, KT * WCOL], BF16, tag="w1t")
                nc.sync.dma_start(wt, w1c[:, g * W1_G:(g + 1) * W1_G, :])
                for t in range(W1_G):
                    s = g * W1_G + t
                    nc.tensor.matmul(pw, lhsT=oT[:, :, s], rhs=wt[:, t, :],
                                     start=(s == 0), stop=(s == SEQ - 1))

            # diagonal extraction: h[n] = sum_a pw[a, 64a+n]; duplicate each
            # diag block in the free dim so one K=4 matmul per a emits h to
            # both partition halves (for the two w2 K-groups).
            hsum = cp.tile([4, KT, P], F32, tag="hsum")
            for a in range(4):
                nc.any.tensor_copy(hsum[:, a, 0:DK], pw[:, a * DK:(a + 1) * DK])
                nc.any.tensor_copy(hsum[:, a, DK:P], pw[:, a * DK:(a + 1) * DK])
            ph = psC.tile([P, 1], F32, tag="tr")
            for a in range(4):
                nc.tensor.matmul(ph, lhsT=hsum[0:4, a, :], rhs=ident[0:4, a:a + 1],
                                 start=(a == 0), stop=(a == 3))
            h2 = cp.tile([P, 1], BF16, tag="h2")
            nc.scalar.activation(h2, ph, AF.Relu, bias=b1_sb)

            # ---- partial z = h_c @ w2_c (this core's 64 rows of w2) ----
            # w2c is host-packed [128, 16000]: partitions 0:64 cover z columns
            # [0,16000), partitions 64:128 cover [16000,32000).  The two
            # halves' LDWEIGHTS run on disjoint row groups -> concurrent.
            pz = [psC.tile([P, ZT], F32, tag="tr", name=f"pz{half}")
                  for half in range(2)]
            for t in range(ZT):
                for half in range(2):
                    lo = half * DK
                    nc.tensor.matmul(pz[half][:, t:t + 1],
                                     lhsT=w2_sb[lo:lo + DK, t * P:(t + 1) * P],
                                     rhs=h2[lo:lo + DK, :],
                                     start=True, stop=True)
            z_sb = cp.tile([P, 2 * ZT], F32, tag="z")
            for half in range(2):
                nc.any.tensor_copy(z_sb[:, half * ZT:(half + 1) * ZT], pz[half])
            nc.sync.dma_start(zout, z_sb)

    nc.compile()
    return nc


_NC_CACHE = {}


def _get_nc():
    if "nc" not in _NC_CACHE:
        _NC_CACHE["nc"] = _build_nc()
    return _NC_CACHE["nc"]


def _pos_encoding_np():
    pos = np.arange(SEQ, dtype=np.float32)[:, None]
    div = np.power(np.float32(10000.0),
                   np.arange(0, HID, 2, dtype=np.float32) / np.float32(HID))
    ang = (pos / div).astype(np.float32)
    pe = np.zeros((SEQ, HID), np.float32)
    pe[:, 0::2] = np.sin(ang)
    pe[:, 1::2] = np.cos(ang)
    return pe


def _pack_inputs(x, emb, wq, wk, wv, w1, b1, w2):
    x0 = np.asarray(x)[0].astype(np.int32)
    xidx = np.ascontiguousarray(x0.reshape(ST, P).T)                     # [128, 2]
    posP = np.ascontiguousarray(
        _pos_encoding_np().reshape(ST, P, HID).transpose(1, 0, 2))       # [128, 2, 512]
    wqkv = np.stack([np.asarray(wq), np.asarray(wk), np.asarray(wv)], axis=1)
    wqkvP = np.ascontiguousarray(
        wqkv.reshape(NL, 3, KT, P, HID).transpose(3, 0, 1, 2, 4)
    ).astype(NP_BF16)                                                    # [128,2,3,4,512]
    embF = np.ascontiguousarray(np.asarray(emb), dtype=np.float32)
    w1F = np.asarray(w1, dtype=np.float32)
    w2F = np.asarray(w2, dtype=np.float32)
    b1F = np.asarray(b1, dtype=np.float32)

    in_maps = []
    for c in range(NCORES):
        w1c = w1F[:, c * WCOL:(c + 1) * WCOL]                            # (131072, 64)
        # [p, s, (a, n)]: w1[512s + 128a + p, n]
        w1P = np.ascontiguousarray(
            w1c.reshape(SEQ, KT, P, WCOL).transpose(2, 0, 1, 3)
            .reshape(P, SEQ, KT * WCOL).astype(NP_BF16))                 # [128,256,256]
        w2r = w2F[c * WCOL:(c + 1) * WCOL]                               # (64, 32000)
        w2P = np.ascontiguousarray(
            np.concatenate([w2r[:, :HALF_OUT], w2r[:, HALF_OUT:]],
                           axis=0).astype(NP_BF16))                      # [128, 16000]
        b1P = np.ascontiguousarray(
            np.tile(b1F[c * WCOL:(c + 1) * WCOL], 2)[:, None])           # [128, 1]
        in_maps.append({
            "xidx": xidx, "emb": embF, "pos": posP, "wqkv": wqkvP,
            "w1c": w1P, "b1c": b1P, "w2c": w2P,
        })
    return in_maps


def run_on_device(in_maps, **kwargs):
    nc = _get_nc()
    return run_bass_kernel_spmd(nc, in_maps, list(range(NCORES)), **kwargs)


def _epilogue(results, b2):
    z = np.zeros(OUT, np.float64)
    for c in range(NCORES):
        zc = np.asarray(results[c]["zout"], dtype=np.float64)            # [128, 250]
        z[:HALF_OUT] += zc[:, :ZT].T.ravel()
        z[HALF_OUT:] += zc[:, ZT:].T.ravel()
    z += np.asarray(b2, dtype=np.float64)
    e = np.exp(z - z.max())
    g = e / e.sum()
    e2 = np.exp(g - g.max())
    probs = e2 / e2.sum()
    return probs.astype(np.float32)


def kernel(x, emb, wq, wk, wv, w1, b1, w2, b2):
    in_maps = _pack_inputs(x, emb, wq, wk, wv, w1, b1, w2)
    res = run_on_device(in_maps)
    probs = _epilogue(res.results, b2)
    return probs, np.float32(0.0)


# revision 20
# speedup vs baseline: 3.1205x; 1.0416x over previous
"""Trainium2 Bass kernel for nn_Network_51445118271910 (moe_routing).

Math (identical to the reference, dead-code-eliminated):
  - The reference returns (probs[0], 0.0); every op in the network is
    batch-independent, so only batch row 0 of x matters.
  - o = emb[x0] + pos_encoding            (256, 512)  on-device gather
  - 2x MHSA, 8 heads, no residual         (256, 512)
  - h = relu(o.flat @ w1 + b1)            (512,)
  - z = h @ w2 + b2                       (32000,)
  - out = softmax(softmax(z))             (host epilogue, 32000 floats)

Sharding over 8 cores (single SPMD launch, no device collectives):
  - attention is replicated (tiny: one sequence),
  - w1 (131072x512) column-sharded: core c computes h[64c:64c+64],
  - w2 (512x32000) row-sharded: core c computes the partial logits
    h[64c:64c+64] @ w2[64c:64c+64, :] -> (32000,),
  - host sums the 8 partial logit vectors, adds b2, double-softmax.

Weights stream in bf16 (ample precision: the double softmax at the end
makes the output insensitive at the 1e-4 level); PSUM accumulation is
fp32 throughout; partial logits return in fp32.
"""

import ml_dtypes
import numpy as np

import concourse.bass as bass
import concourse.mybir as mybir
import concourse.tile as tile
from concourse import bacc
from concourse.bass_utils import run_bass_kernel_spmd
from concourse.masks import make_identity

P = 128
SEQ = 256
HID = 512
NH = 8
DK = 64
NL = 2
VOCAB = 16384
OUT = 32000
KT = HID // P          # 4 k-tiles of the hidden dim
ST = SEQ // P          # 2 tiles of the sequence dim
NCORES = 8
WCOL = 64              # w1 columns / w2 rows per core
HALF_OUT = OUT // 2    # 16000
ZT = HALF_OUT // P     # 125 z column-blocks per half
NPAIR = SEQ // 2       # 128 token pairs for the w1 contraction
W1_G = 8               # token-pairs per streamed w1 tile (8*512*2B*128 = 1MB)
W1_BUFS = 20
F32 = mybir.dt.float32
BF16 = mybir.dt.bfloat16
AF = mybir.ActivationFunctionType
NP_BF16 = ml_dtypes.bfloat16


def _build_nc():
    nc = bacc.Bacc("TRN2", target_bir_lowering=False, debug=False,
                   num_devices=NCORES)

    xidx = nc.dram_tensor("xidx", [P, ST], mybir.dt.int32, kind="ExternalInput").ap()
    emb = nc.dram_tensor("emb", [VOCAB, HID], F32, kind="ExternalInput").ap()
    pos = nc.dram_tensor("pos", [P, ST, HID], F32, kind="ExternalInput").ap()
    wqkv = nc.dram_tensor("wqkv", [P, NL, 3, KT, HID], BF16, kind="ExternalInput").ap()
    w1c = nc.dram_tensor("w1c", [P, SEQ, KT * WCOL], BF16,
                         kind="ExternalInput").ap()
    b1c = nc.dram_tensor("b1c", [P, 1], F32, kind="ExternalInput").ap()
    w2c = nc.dram_tensor("w2c", [P, HALF_OUT], BF16, kind="ExternalInput").ap()
    zout = nc.dram_tensor("zout", [P, 2 * ZT], F32, kind="ExternalOutput").ap()

    with tile.TileContext(nc) as tc:
        with (
            tc.tile_pool(name="cp", bufs=1) as cp,          # constants / persistent
            tc.tile_pool(name="op", bufs=2) as op,          # o / oT activations
            tc.tile_pool(name="qk", bufs=1) as qk,          # qT / kT / v
            tc.tile_pool(name="ep", bufs=2) as ep,          # exp(scores^T) per head
            tc.tile_pool(name="rp", bufs=4) as rp,          # tiny per-head scalars
            tc.tile_pool(name="w1p", bufs=W1_BUFS) as w1p,  # streamed w1 tiles
            tc.tile_pool(name="psA", bufs=4, space="PSUM") as psA,   # matmul banks
            tc.tile_pool(name="psB", bufs=2, space="PSUM") as psB,   # attention out
            tc.tile_pool(name="psC", bufs=2, space="PSUM") as psC,   # transposes / z
        ):
            # ---- small inputs + gather first (critical path to attention) ----
            x_sb = cp.tile([P, ST], mybir.dt.int32, tag="x")
            nc.sync.dma_start(x_sb, xidx)
            b1_sb = cp.tile([P, 1], F32, tag="b1")
            nc.sync.dma_start(b1_sb, b1c)

            w_sb = {}
            for l in range(NL):
                for m in range(3):
                    t = cp.tile([P, KT, HID], BF16, tag=f"w{l}{m}")
                    nc.sync.dma_start(t, wqkv[:, l, m])
                    w_sb[(l, m)] = t

            # w2 resident (bf16, 4MB); streamed on the ACT HW-DGE ring so it
            # overlaps the w1 stream on the sync ring.
            w2_sb = cp.tile([P, HALF_OUT], BF16, tag="w2")
            for i in range(4):
                sl = slice(i * HALF_OUT // 4, (i + 1) * HALF_OUT // 4)
                nc.scalar.dma_start(w2_sb[:, sl], w2c[:, sl])

            # ---- embedding gather + positional encoding ----
            # pos is DMAed into o0 first, then the gather accumulates emb[x0]
            # on top of it (CCE add in the DMA engine).
            o0 = op.tile([P, ST, HID], F32, tag="o")
            nc.gpsimd.dma_start(o0, pos)
            for i in range(ST):
                nc.gpsimd.indirect_dma_start(
                    out=o0[:, i, :], out_offset=None, in_=emb,
                    in_offset=bass.IndirectOffsetOnAxis(ap=x_sb[:, i:i + 1], axis=0),
                    compute_op=mybir.AluOpType.add,
                )

            ident = cp.tile([P, P], F32, tag="ident")
            make_identity(nc, ident)
            identb = cp.tile([P, P], BF16, tag="identb")
            make_identity(nc, identb)

            def transpose_sd_to_ds(o_sd, oT_ds, idt):
                # [s-part, st, d] -> [d-part, dt, s] via 8 PE transposes
                for dt in range(KT):
                    for st in range(ST):
                        pt = psC.tile([P, P], o_sd.dtype, tag="tr")
                        nc.tensor.transpose(pt, o_sd[:, st, dt * P:(dt + 1) * P], idt)
                        nc.any.tensor_copy(oT_ds[:, dt, st * P:(st + 1) * P], pt)

            oT = op.tile([P, KT, SEQ], BF16, tag="oT")
            transpose_sd_to_ds(o0, oT, ident)

            # ---- 2 MHSA layers (bf16 operands, fp32 PSUM) ----
            for l in range(NL):
                qT = qk.tile([P, KT, SEQ], BF16, tag="qT")
                kT = qk.tile([P, KT, SEQ], BF16, tag="kT")
                # v with a ones-column per head (for the softmax denominator)
                v_sb = qk.tile([P, ST, NH, DK + 1], BF16, tag="v")
                nc.gpsimd.memset(v_sb[:, :, :, DK:DK + 1], 1.0)

                for m, dst in ((0, qT), (1, kT)):
                    for jt in range(KT):
                        pq = psA.tile([P, 512], F32, tag="mm")
                        for kt in range(KT):
                            nc.tensor.matmul(
                                pq[:, :SEQ],
                                lhsT=w_sb[(l, m)][:, kt, jt * P:(jt + 1) * P],
                                rhs=oT[:, kt, :],
                                start=(kt == 0), stop=(kt == KT - 1),
                            )
                        nc.any.tensor_copy(dst[:, jt, :], pq[:, :SEQ])

                for tt in range(ST):
                    pv = psA.tile([P, 512], F32, tag="mm")
                    for kt in range(KT):
                        nc.tensor.matmul(
                            pv,
                            lhsT=oT[:, kt, tt * P:(tt + 1) * P],
                            rhs=w_sb[(l, 2)][:, kt, :],
                            start=(kt == 0), stop=(kt == KT - 1),
                        )
                    for h in range(NH):
                        nc.any.tensor_copy(v_sb[:, tt, h, 0:DK],
                                           pv[:, h * DK:(h + 1) * DK])

                o_out = op.tile([P, ST, HID], BF16, tag="ob")
                for h in range(NH):
                    jt, lo = h // 2, (h % 2) * DK
                    qTh = qT[lo:lo + DK, jt, :]
                    kTh = kT[lo:lo + DK, jt, :]
                    eT = ep.tile([P, ST, SEQ], BF16, tag="expT")
                    for tt in range(ST):
                        ps = psA.tile([P, 512], F32, tag="mm")
                        nc.tensor.matmul(ps[:, :SEQ],
                                         lhsT=kTh[:, tt * P:(tt + 1) * P],
                                         rhs=qTh, start=True, stop=True)
                        # softmax without max-subtraction: |scores|/8 <= ~2.2
                        nc.scalar.activation(eT[:, tt, :], ps[:, :SEQ],
                                             AF.Exp, scale=0.125)
                    for st in range(ST):
                        pa = psB.tile([P, DK + 1], F32, tag="att")
                        for tt in range(ST):
                            nc.tensor.matmul(pa,
                                             lhsT=eT[:, tt, st * P:(st + 1) * P],
                                             rhs=v_sb[:, tt, h, :],
                                             start=(tt == 0), stop=(tt == ST - 1))
                        rec = rp.tile([P, 1], F32, tag="rec")
                        nc.vector.reciprocal(rec, pa[:, DK:DK + 1])
                        nc.vector.tensor_scalar_mul(
                            o_out[:, st, h * DK:(h + 1) * DK], pa[:, 0:DK], rec)

                oT = op.tile([P, KT, SEQ], BF16, tag="oT")
                transpose_sd_to_ds(o_out, oT, identb)

            # ---- h = relu(flat @ w1 + b1), this core's 64 columns ----
            # per token s: psum[a, 64a+n] += sum_p oT[p, a, s] * w1[512s+128a+p, n]
            pw = psA.tile([4, KT * WCOL], F32, tag="mm")
            for g in range(SEQ // W1_G):
                wt = w1p.tile([P, W1_G, KT * WCOL], BF16, tag="w1t")
                nc.sync.dma_start(wt, w1c[:, g * W1_G:(g + 1) * W1_G, :])
                for t in range(W1_G):
                    s = g * W1_G + t
                    nc.tensor.matmul(pw, lhsT=oT[:, :, s], rhs=wt[:, t, :],
                                     start=(s == 0), stop=(s == SEQ - 1))

            # diagonal extraction: h[n] = sum_a pw[a, 64a+n]; duplicate each
            # diag block in the free dim so one K=4 matmul per a emits h to
            # both partition halves (for the two w2 K-groups).
            hsum = cp.tile([4, KT, P], F32, tag="hsum")
            for a in range(4):
                nc.any.tensor_copy(hsum[:, a, 0:DK], pw[:, a * DK:(a + 1) * DK])
                nc.any.tensor_copy(hsum[:, a, DK:P], pw[:, a * DK:(a + 1) * DK])
            ph = psC.tile([P, 1], F32, tag="tr")
            for a in range(4):
                nc.tensor.matmul(ph, lhsT=hsum[0:4, a, :], rhs=ident[0:4, a:a + 1],
                                 start=(a == 0), stop=(a == 3))
            h2 = cp.tile([P, 1], BF16, tag="h2")
            nc.scalar.activation(h2, ph, AF.Relu, bias=b1_sb)

            # ---- partial z = h_c @ w2_c (this core's 64 rows of w2) ----
            # w2c is host-packed [128, 16000]: partitions 0:64 cover z columns
            # [0,16000), partitions 64:128 cover [16000,32000).  The two
            # halves' LDWEIGHTS run on disjoint row groups -> concurrent.
            pz = [psC.tile([P, ZT], F32, tag="tr", name=f"pz{half}")
                  for half in range(2)]
            for t in range(ZT):
                for half in range(2):
                    lo = half * DK
                    nc.tensor.matmul(pz[half][:, t:t + 1],
                                     lhsT=w2_sb[lo:lo + DK, t * P:(t + 1) * P],
                                     rhs=h2[lo:lo + DK, :],
                                     start=True, stop=True)
            z_sb = cp.tile([P, 2 * ZT], F32, tag="z")
            for half in range(2):
                nc.any.tensor_copy(z_sb[:, half * ZT:(half + 1) * ZT], pz[half])
            nc.sync.dma_start(zout, z_sb)

    nc.compile()
    return nc


_NC_CACHE = {}


def _get_nc():
    if "nc" not in _NC_CACHE:
        _NC_CACHE["nc"] = _build_nc()
    return _NC_CACHE["nc"]


def _pos_encoding_np():
    pos = np.arange(SEQ, dtype=np.float32)[:, None]
    div = np.power(np.float32(10000.0),
                   np.arange(0, HID, 2, dtype=np.float32) / np.float32(HID))
    ang = (pos / div).astype(np.float32)
    pe = np.zeros((SEQ, HID), np.float32)
    pe[:, 0::2] = np.sin(ang)
    pe[:, 1::2] = np.cos(ang)
    return pe


def _pack_inputs(x, emb, wq, wk, wv, w1, b1, w2):
    x0 = np.asarray(x)[0].astype(np.int32)
    xidx = np.ascontiguousarray(x0.reshape(ST, P).T)                     # [128, 2]
    posP = np.ascontiguousarray(
        _pos_encoding_np().reshape(ST, P, HID).transpose(1, 0, 2))       # [128, 2, 512]
    wqkv = np.stack([np.asarray(wq), np.asarray(wk), np.asarray(wv)], axis=1)
    wqkvP = np.ascontiguousarray(
        wqkv.reshape(NL, 3, KT, P, HID).transpose(3, 0, 1, 2, 4)
    ).astype(NP_BF16)                                                    # [128,2,3,4,512]
    embF = np.ascontiguousarray(np.asarray(emb), dtype=np.float32)
    w1F = np.asarray(w1, dtype=np.float32)
    w2F = np.asarray(w2, dtype=np.float32)
    b1F = np.asarray(b1, dtype=np.float32)

    in_maps = []
    for c in range(NCORES):
        w1c = w1F[:, c * WCOL:(c + 1) * WCOL]                            # (131072, 64)
        # [p, s, (a, n)]: w1[512s + 128a + p, n]
        w1P = np.ascontiguousarray(
            w1c.reshape(SEQ, KT, P, WCOL).transpose(2, 0, 1, 3)
            .reshape(P, SEQ, KT * WCOL).astype(NP_BF16))                 # [128,256,256]
        w2r = w2F[c * WCOL:(c + 1) * WCOL]                               # (64, 32000)
        w2P = np.ascontiguousarray(
            np.concatenate([w2r[:, :HALF_OUT], w2r[:, HALF_OUT:]],
                           axis=0).astype(NP_BF16))                      # [128, 16000]
        b1P = np.ascontiguousarray(
            np.tile(b1F[c * WCOL:(c + 1) * WCOL], 2)[:, None])           # [128, 1]
        in_maps.append({
            "xidx": xidx, "emb": embF, "pos": posP, "wqkv": wqkvP,
            "w1c": w1P, "b1c": b1P, "w2c": w2P,
        })
    return in_maps


def run_on_device(in_maps, **kwargs):
    nc = _get_nc()
    return run_bass_kernel_spmd(nc, in_maps, list(range(NCORES)), **kwargs)


def _epilogue(results, b2):
    z = np.zeros(OUT, np.float64)
    for c in range(NCORES):
        zc = np.asarray(results[c]["zout"], dtype=np.float64)            # [128, 250]
        z[:HALF_OUT] += zc[:, :ZT].T.ravel()
        z[HALF_OUT:] += zc[:, ZT:].T.ravel()
    z += np.asarray(b2, dtype=np.float64)
    e = np.exp(z - z.max())
    g = e / e.sum()
    e2 = np.exp(g - g.max())
    probs = e2 / e2.sum()
    return probs.astype(np.float32)


def kernel(x, emb, wq, wk, wv, w1, b1, w2, b2):
    in_maps = _pack_inputs(x, emb, wq, wk, wv, w1, b1, w2)
    res = run_on_device(in_maps)
    probs = _epilogue(res.results, b2)
    return probs, np.float32(0.0)


# revision 29
# speedup vs baseline: 3.1902x; 1.0223x over previous
"""Trainium2 Bass kernel for nn_Network_51445118271910 (moe_routing).

Math (identical to the reference, dead-code-eliminated):
  - The reference returns (probs[0], 0.0); every op in the network is
    batch-independent, so only batch row 0 of x matters.
  - o = emb[x0] + pos_encoding            (256, 512)  on-device gather
  - 2x MHSA, 8 heads, no residual         (256, 512)
  - h = relu(o.flat @ w1 + b1)            (512,)
  - z = h @ w2 + b2                       (32000,)
  - out = softmax(softmax(z))             (host epilogue, 32000 floats)

Sharding over 8 cores (single SPMD launch, no device collectives):
  - attention is replicated (tiny: one sequence),
  - w1 (131072x512) column-sharded: core c computes h[64c:64c+64],
  - w2 (512x32000) row-sharded: core c computes the partial logits
    h[64c:64c+64] @ w2[64c:64c+64, :] -> (32000,),
  - host sums the 8 partial logit vectors, adds b2, double-softmax.

Weights stream in bf16 (ample precision: the double softmax at the end
makes the output insensitive at the 1e-4 level); PSUM accumulation is
fp32 throughout; partial logits return in fp32.
"""

import ml_dtypes
import numpy as np

import concourse.bass as bass
import concourse.mybir as mybir
import concourse.tile as tile
from concourse import bacc
from concourse.bass_utils import run_bass_kernel_spmd
from concourse.masks import make_identity

P = 128
SEQ = 256
HID = 512
NH = 8
DK = 64
NL = 2
VOCAB = 16384
OUT = 32000
KT = HID // P          # 4 k-tiles of the hidden dim
ST = SEQ // P          # 2 tiles of the sequence dim
NCORES = 8
WCOL = 64              # w1 columns / w2 rows per core
HALF_OUT = OUT // 2    # 16000
ZT = HALF_OUT // P     # 125 z column-blocks per half
NPAIR = SEQ // 2       # 128 token pairs for the w1 contraction
W1_G = 8               # token-pairs per streamed w1 tile (8*512*2B*128 = 1MB)
W1_BUFS = 28
F32 = mybir.dt.float32
BF16 = mybir.dt.bfloat16
FP8 = mybir.dt.float8e4
AF = mybir.ActivationFunctionType
NP_BF16 = ml_dtypes.bfloat16
NP_FP8 = ml_dtypes.float8_e4m3
# fp8 scale factors: operands are scaled up into e4m3's sweet spot and the
# products scaled back down (w1 result in the relu, w2 result on the host)
S_ACT = 16.0   # layer-2 activations (|o2| <= ~0.3)
S_W1 = 16.0
S_H = 8.0      # h (<= ~2.1)
S_W2 = 16.0
Z_DESCALE = S_H * S_W2


def _build_nc():
    nc = bacc.Bacc("TRN2", target_bir_lowering=False, debug=False,
                   num_devices=NCORES)

    xidx = nc.dram_tensor("xidx", [P, ST], mybir.dt.int32, kind="ExternalInput").ap()
    emb = nc.dram_tensor("emb", [VOCAB, HID], F32, kind="ExternalInput").ap()
    pos = nc.dram_tensor("pos", [P, ST, HID], F32, kind="ExternalInput").ap()
    wqkv = nc.dram_tensor("wqkv", [P, NL, 3, KT, HID], BF16, kind="ExternalInput").ap()
    w1c = nc.dram_tensor("w1c", [P, SEQ, KT * WCOL], FP8,
                         kind="ExternalInput").ap()
    b1c = nc.dram_tensor("b1c", [P, 1], F32, kind="ExternalInput").ap()
    w2c = nc.dram_tensor("w2c", [P, HALF_OUT], FP8, kind="ExternalInput").ap()
    zout = nc.dram_tensor("zout", [P, 2 * ZT], F32, kind="ExternalOutput").ap()

    with tile.TileContext(nc) as tc:
        with (
            tc.tile_pool(name="cp", bufs=1) as cp,          # constants / persistent
            tc.tile_pool(name="op", bufs=2) as op,          # o / oT activations
            tc.tile_pool(name="qk", bufs=1) as qk,          # qT / kT / v
            tc.tile_pool(name="ep", bufs=2) as ep,          # exp(scores^T) per head
            tc.tile_pool(name="rp", bufs=4) as rp,          # tiny per-head scalars
            tc.tile_pool(name="w1p", bufs=W1_BUFS) as w1p,  # streamed w1 tiles
            tc.tile_pool(name="psA", bufs=4, space="PSUM") as psA,   # matmul banks
            tc.tile_pool(name="psB", bufs=2, space="PSUM") as psB,   # attention out
            tc.tile_pool(name="psC", bufs=2, space="PSUM") as psC,   # transposes / z
        ):
            # ---- small inputs + gather first (critical path to attention) ----
            x_sb = cp.tile([P, ST], mybir.dt.int32, tag="x")
            nc.sync.dma_start(x_sb, xidx)
            b1_sb = cp.tile([P, 1], F32, tag="b1")
            nc.sync.dma_start(b1_sb, b1c)

            w_sb = {}
            for l in range(NL):
                for m in range(3):
                    t = cp.tile([P, KT, HID], BF16, tag=f"w{l}{m}")
                    nc.sync.dma_start(t, wqkv[:, l, m])
                    w_sb[(l, m)] = t

            # w2 resident (bf16, 4MB); streamed on the ACT HW-DGE ring so it
            # overlaps the w1 stream on the sync ring.
            w2_sb = cp.tile([P, HALF_OUT], FP8, tag="w2")
            for i in range(4):
                sl = slice(i * HALF_OUT // 4, (i + 1) * HALF_OUT // 4)
                nc.scalar.dma_start(w2_sb[:, sl], w2c[:, sl])

            # ---- embedding gather + positional encoding ----
            # pos is DMAed into o0 first, then the gather accumulates emb[x0]
            # on top of it (CCE add in the DMA engine).
            o0 = op.tile([P, ST, HID], F32, tag="o")
            nc.gpsimd.dma_start(o0, pos)
            for i in range(ST):
                nc.gpsimd.indirect_dma_start(
                    out=o0[:, i, :], out_offset=None, in_=emb,
                    in_offset=bass.IndirectOffsetOnAxis(ap=x_sb[:, i:i + 1], axis=0),
                    compute_op=mybir.AluOpType.add,
                )

            ident = cp.tile([P, P], F32, tag="ident")
            make_identity(nc, ident)
            identb = cp.tile([P, P], BF16, tag="identb")
            make_identity(nc, identb)

            def transpose_sd_to_ds(o_sd, oT_ds, idt, scale=None):
                # [s-part, st, d] -> [d-part, dt, s] via 8 PE transposes
                for dt in range(KT):
                    for st in range(ST):
                        pt = psC.tile([P, P], o_sd.dtype, tag="tr")
                        nc.tensor.transpose(pt, o_sd[:, st, dt * P:(dt + 1) * P], idt)
                        dst = oT_ds[:, dt, st * P:(st + 1) * P]
                        if scale is None:
                            nc.any.tensor_copy(dst, pt)
                        else:
                            nc.vector.tensor_scalar_mul(dst, pt, scale)

            oT = op.tile([P, KT, SEQ], BF16, tag="oT")
            transpose_sd_to_ds(o0, oT, ident)

            # ---- 2 MHSA layers (bf16 operands, fp32 PSUM) ----
            for l in range(NL):
                qT = qk.tile([P, KT, SEQ], BF16, tag="qT")
                kT = qk.tile([P, KT, SEQ], BF16, tag="kT")
                # v with a ones-column per head (for the softmax denominator)
                v_sb = qk.tile([P, ST, NH, DK + 1], BF16, tag="v")
                nc.gpsimd.memset(v_sb[:, :, :, DK:DK + 1], 1.0)

                for m, dst in ((0, qT), (1, kT)):
                    for jt in range(KT):
                        pq = psA.tile([P, 512], F32, tag="mm")
                        for kt in range(KT):
                            nc.tensor.matmul(
                                pq[:, :SEQ],
                                lhsT=w_sb[(l, m)][:, kt, jt * P:(jt + 1) * P],
                                rhs=oT[:, kt, :],
                                start=(kt == 0), stop=(kt == KT - 1),
                            )
                        nc.any.tensor_copy(dst[:, jt, :], pq[:, :SEQ])

                for tt in range(ST):
                    pv = psA.tile([P, 512], F32, tag="mm")
                    for kt in range(KT):
                        nc.tensor.matmul(
                            pv,
                            lhsT=oT[:, kt, tt * P:(tt + 1) * P],
                            rhs=w_sb[(l, 2)][:, kt, :],
                            start=(kt == 0), stop=(kt == KT - 1),
                        )
                    for h in range(NH):
                        nc.any.tensor_copy(v_sb[:, tt, h, 0:DK],
                                           pv[:, h * DK:(h + 1) * DK])

                o_out = op.tile([P, ST, HID], BF16, tag="ob")
                for h in range(NH):
                    jt, lo = h // 2, (h % 2) * DK
                    qTh = qT[lo:lo + DK, jt, :]
                    kTh = kT[lo:lo + DK, jt, :]
                    eT = ep.tile([P, ST, SEQ], BF16, tag="expT")
                    for tt in range(ST):
                        ps = psA.tile([P, 512], F32, tag="mm")
                        nc.tensor.matmul(ps[:, :SEQ],
                                         lhsT=kTh[:, tt * P:(tt + 1) * P],
                                         rhs=qTh, start=True, stop=True)
                        # softmax without max-subtraction: |scores|/8 <= ~2.2
                        nc.scalar.activation(eT[:, tt, :], ps[:, :SEQ],
                                             AF.Exp, scale=0.125)
                    for st in range(ST):
                        pa = psB.tile([P, DK + 1], F32, tag="att")
                        for tt in range(ST):
                            nc.tensor.matmul(pa,
                                             lhsT=eT[:, tt, st * P:(st + 1) * P],
                                             rhs=v_sb[:, tt, h, :],
                                             start=(tt == 0), stop=(tt == ST - 1))
                        rec = rp.tile([P, 1], F32, tag="rec")
                        nc.vector.reciprocal(rec, pa[:, DK:DK + 1])
                        nc.vector.tensor_scalar_mul(
                            o_out[:, st, h * DK:(h + 1) * DK], pa[:, 0:DK], rec)

                if l < NL - 1:
                    oT = op.tile([P, KT, SEQ], BF16, tag="oT")
                    transpose_sd_to_ds(o_out, oT, identb)
                else:
                    # the final oT feeds only the fp8 w1 contraction: write it
                    # as fp8 scaled by S_ACT directly from the transpose psum
                    oT = op.tile([P, KT, SEQ], FP8, tag="oT8")
                    transpose_sd_to_ds(o_out, oT, identb, scale=S_ACT)

            # ---- h = relu(flat @ w1 + b1), this core's 64 columns ----
            # per token s: psum[a, 64a+n] += sum_p oT[p, a, s] * w1[512s+128a+p, n]
            pw = psA.tile([4, KT * WCOL], F32, tag="mm")
            for g in range(SEQ // W1_G):
                wt = w1p.tile([P, W1_G, KT * WCOL], FP8, tag="w1t")
                nc.sync.dma_start(wt, w1c[:, g * W1_G:(g + 1) * W1_G, :])
                for t in range(W1_G):
                    s = g * W1_G + t
                    nc.tensor.matmul(pw, lhsT=oT[:, :, s], rhs=wt[:, t, :],
                                     start=(s == 0), stop=(s == SEQ - 1))

            # diagonal extraction: h[n] = sum_a pw[a, 64a+n]; duplicate each
            # diag block in the free dim so one K=4 matmul per a emits h to
            # both partition halves (for the two w2 K-groups).
            hsum = cp.tile([4, KT, P], F32, tag="hsum")
            for a in range(4):
                nc.any.tensor_copy(hsum[:, a, 0:DK], pw[:, a * DK:(a + 1) * DK])
                nc.any.tensor_copy(hsum[:, a, DK:P], pw[:, a * DK:(a + 1) * DK])
            ph = psC.tile([P, 1], F32, tag="tr")
            for a in range(4):
                nc.tensor.matmul(ph, lhsT=hsum[0:4, a, :], rhs=ident[0:4, a:a + 1],
                                 start=(a == 0), stop=(a == 3))
            # h = relu(psum/(S_ACT*S_W1) + b1), then rescale by S_H into fp8
            hf = cp.tile([P, 1], F32, tag="hf")
            nc.scalar.activation(hf, ph, AF.Relu, bias=b1_sb,
                                 scale=1.0 / (S_ACT * S_W1))
            h2 = cp.tile([P, 1], FP8, tag="h2")
            nc.vector.tensor_scalar_mul(h2, hf, S_H)

            # ---- partial z = h_c @ w2_c (this core's 64 rows of w2) ----
            # w2c is host-packed [128, 16000]: partitions 0:64 cover z columns
            # [0,16000), partitions 64:128 cover [16000,32000).  The two
            # halves' LDWEIGHTS run on disjoint row groups -> concurrent.
            pz = [psC.tile([P, ZT], F32, tag="tr", name=f"pz{half}")
                  for half in range(2)]
            for t in range(ZT):
                for half in range(2):
                    lo = half * DK
                    nc.tensor.matmul(pz[half][:, t:t + 1],
                                     lhsT=w2_sb[lo:lo + DK, t * P:(t + 1) * P],
                                     rhs=h2[lo:lo + DK, :],
                                     start=True, stop=True)
            z_sb = cp.tile([P, 2 * ZT], F32, tag="z")
            for half in range(2):
                nc.any.tensor_copy(z_sb[:, half * ZT:(half + 1) * ZT], pz[half])
            nc.sync.dma_start(zout, z_sb)

    nc.compile()
    return nc


_NC_CACHE = {}


def _get_nc():
    if "nc" not in _NC_CACHE:
        _NC_CACHE["nc"] = _build_nc()
    return _NC_CACHE["nc"]


def _pos_encoding_np():
    pos = np.arange(SEQ, dtype=np.float32)[:, None]
    div = np.power(np.float32(10000.0),
                   np.arange(0, HID, 2, dtype=np.float32) / np.float32(HID))
    ang = (pos / div).astype(np.float32)
    pe = np.zeros((SEQ, HID), np.float32)
    pe[:, 0::2] = np.sin(ang)
    pe[:, 1::2] = np.cos(ang)
    return pe


def _pack_inputs(x, emb, wq, wk, wv, w1, b1, w2):
    x0 = np.asarray(x)[0].astype(np.int32)
    xidx = np.ascontiguousarray(x0.reshape(ST, P).T)                     # [128, 2]
    posP = np.ascontiguousarray(
        _pos_encoding_np().reshape(ST, P, HID).transpose(1, 0, 2))       # [128, 2, 512]
    wqkv = np.stack([np.asarray(wq), np.asarray(wk), np.asarray(wv)], axis=1)
    wqkvP = np.ascontiguousarray(
        wqkv.reshape(NL, 3, KT, P, HID).transpose(3, 0, 1, 2, 4)
    ).astype(NP_BF16)                                                    # [128,2,3,4,512]
    embF = np.ascontiguousarray(np.asarray(emb), dtype=np.float32)
    w1F = np.asarray(w1, dtype=np.float32)
    w2F = np.asarray(w2, dtype=np.float32)
    b1F = np.asarray(b1, dtype=np.float32)

    in_maps = []
    for c in range(NCORES):
        w1c = w1F[:, c * WCOL:(c + 1) * WCOL] * np.float32(S_W1)         # (131072, 64)
        # [p, s, (a, n)]: w1[512s + 128a + p, n]
        w1P = np.ascontiguousarray(
            w1c.reshape(SEQ, KT, P, WCOL).transpose(2, 0, 1, 3)
            .reshape(P, SEQ, KT * WCOL).astype(NP_FP8))                  # [128,256,256]
        w2r = w2F[c * WCOL:(c + 1) * WCOL] * np.float32(S_W2)            # (64, 32000)
        w2P = np.ascontiguousarray(
            np.concatenate([w2r[:, :HALF_OUT], w2r[:, HALF_OUT:]],
                           axis=0).astype(NP_FP8))                       # [128, 16000]
        b1P = np.ascontiguousarray(
            np.tile(b1F[c * WCOL:(c + 1) * WCOL], 2)[:, None])           # [128, 1]
        in_maps.append({
            "xidx": xidx, "emb": embF, "pos": posP, "wqkv": wqkvP,
            "w1c": w1P, "b1c": b1P, "w2c": w2P,
        })
    return in_maps


def run_on_device(in_maps, **kwargs):
    nc = _get_nc()
    return run_bass_kernel_spmd(nc, in_maps, list(range(NCORES)), **kwargs)


def _epilogue(results, b2):
    z = np.zeros(OUT, np.float64)
    for c in range(NCORES):
        zc = np.asarray(results[c]["zout"], dtype=np.float64)            # [128, 250]
        z[:HALF_OUT] += zc[:, :ZT].T.ravel()
        z[HALF_OUT:] += zc[:, ZT:].T.ravel()
    z /= Z_DESCALE
    z += np.asarray(b2, dtype=np.float64)
    e = np.exp(z - z.max())
    g = e / e.sum()
    e2 = np.exp(g - g.max())
    probs = e2 / e2.sum()
    return probs.astype(np.float32)


def kernel(x, emb, wq, wk, wv, w1, b1, w2, b2):
    in_maps = _pack_inputs(x, emb, wq, wk, wv, w1, b1, w2)
    res = run_on_device(in_maps)
    probs = _epilogue(res.results, b2)
    return probs, np.float32(0.0)
